# revision 6
# baseline (speedup 1.0000x reference)
"""BitNetLinear forward on 8 TRN2 NeuronCores — fp8 DoubleRow version.

out = x @ (alpha * clip(round(W/alpha), -1, 1))^T
  x [4, 2048, 4096] f32, W [4096, 4096] f32, alpha scalar f32.

Strategy: hybrid 4x2 tensor-parallel — 4 row-groups over the 8192 x-rows
x 2 column-groups over the 4096 out-features. Each core computes a
[2048, 2048] out tile from its x shard [2048, 4096] and W shard
[2048, 4096]. This halves the per-core W traffic AND the per-core
ternarization (ScalarE sign) work vs pure data-parallel, which is what
lets the fp8 PE stream run unstalled. No collectives. Host side only
reshapes/slices (layout); all arithmetic (ternary quantization + fp8
casts + matmul + alpha scaling) runs on device.

Device kernel (per core) — all-fp8 split-K with hi/lo error compensation:
  - W^T streamed in f32, ternarized on the fly to fp8e4 via
    T' = Sign(w + a/2) + Sign(w - a/2) in {-2, 0, 2} (exact in fp8).
  - x^T shard resident in SBUF as fp8e4 "hi" = fp8(x) for all K, plus a
    "lo" residual fp8(x - hi) for the upper half of K (k-tiles 16..31).
    Single-fp8 halves the matmul work vs bf16 (DoubleRow contracts 256/MM
    at the same 512-col stream rate); the hi+lo half restores accuracy
    there, yielding rel err ~1.66e-2 (< 2e-2) at 0.75x the bf16 PE time.
  - Per psum group: 16 hi DR-matmuls (K pairs 0..31) + 8 lo DR-matmuls
    (K pairs 16..31) accumulate into one PSUM bank; evicted through
    ScalarE with scale = alpha/2 (undoes the {-2,0,2} doubling and
    applies the alpha weight scale), DMA to out on ACT's HWDGE ring.
"""

import contextlib
import sys

if "/opt/trn_rl_repo" not in sys.path:
    sys.path.insert(0, "/opt/trn_rl_repo")

import numpy as np

import concourse.bass as bass  # noqa: F401
import concourse.mybir as mybir
import concourse.tile as tile
from concourse import bacc
from concourse.bass_utils import run_bass_kernel_spmd

P = 128
N_CORES = 8
D_IN = 4096  # contraction
D_OUT = 4096
M_TOT = 4 * 2048
RG = 4  # row groups (x-row parallel)
CG = 2  # col groups (out-feature parallel)
M_SHARD = M_TOT // RG  # 2048 rows per core
N_SHARD = D_OUT // CG  # 2048 out-features per core
KO = D_IN // P  # 32 k-tiles
KO_LO = KO // 2  # k-tiles 16..31 get the lo residual pass
N_TILE = 512

F32 = mybir.dt.float32
BF16 = mybir.dt.bfloat16
FP8 = mybir.dt.float8e4
DR = mybir.MatmulPerfMode.DoubleRow


def build(m_shard=M_SHARD, d_in=D_IN, d_out=N_SHARD, reps=1, mode="full",
          wstage_bufs=3, wchunk=4, outs_bufs=4):
    """mode: 'full' (real kernel), 'pe' (timing probe: no input DMA/quant),
    'dma' (W DMA + quant, no matmuls), 'dmax' (x DMA only)."""
    ko = d_in // P
    ko_lo = ko // 2
    n_tiles = d_out // N_TILE
    m_sub = m_shard // P
    xg = max(1, m_shard // 256)
    xw = m_shard // xg

    nc = bacc.Bacc("TRN2", target_bir_lowering=False, debug=False,
                   num_devices=N_CORES)
    xt_d = nc.declare_dram_parameter("xt", [P, ko, m_shard], F32, isOutput=False)
    wt_d = nc.declare_dram_parameter("wt", [P, ko, d_out], F32, isOutput=False)
    al_d = nc.declare_dram_parameter("alpha", [1, 1], F32, isOutput=False)
    out_d = nc.declare_dram_parameter("out", [P, m_sub, d_out], F32, isOutput=True)

    with tile.TileContext(nc) as tc:
        with (
            tc.tile_pool(name="const", bufs=1) as const,
            tc.tile_pool(name="xres", bufs=1) as xres_pool,
            tc.tile_pool(name="stage", bufs=3) as stage,
            tc.tile_pool(name="wstage", bufs=wstage_bufs) as wstage,
            tc.tile_pool(name="wq", bufs=2) as wqp,
            tc.tile_pool(name="s2", bufs=1) as s2p,
            tc.tile_pool(name="outs", bufs=outs_bufs) as outs,
            tc.tile_pool(name="psum", bufs=8, space="PSUM") as psum,
        ):
            pe_init = {}
            if mode == "pe":
                pe_init["xres"] = [
                    xres_pool.tile([P, ko, xw], FP8, tag=f"xres{g}",
                                   name=f"xres{g}")
                    for g in range(xg)
                ]
                pe_init["xlo"] = [
                    xres_pool.tile([P, ko_lo, xw], FP8, tag=f"xlo{g}",
                                   name=f"xlo{g}")
                    for g in range(xg)
                ]
                for g in range(xg):
                    nc.vector.memset(pe_init["xres"][g][:], 0.0)
                    nc.vector.memset(pe_init["xlo"][g][:], 0.0)
                pe_init["wq"] = wqp.tile([P, ko, N_TILE], FP8, tag="wq",
                                         name="wq_static")
                nc.vector.memset(pe_init["wq"][:], 0.0)

            rep_ctx = (
                tc.For_i(0, reps, 1) if reps > 1 else contextlib.nullcontext()
            )
            with rep_ctx:
                # alpha -> [1,1] -> broadcast to [128,1]; +a/2 and -a/2.
                a1 = const.tile([1, 1], F32)
                nc.sync.dma_start(out=a1[:, :], in_=al_d.ap()[:, :])
                ab = const.tile([P, 1], F32)
                nc.gpsimd.partition_broadcast(ab[:, :], a1[:, :])
                half = const.tile([P, 1], F32)
                nc.vector.tensor_scalar_mul(half[:, :], ab[:, :], 0.5)
                neghalf = const.tile([P, 1], F32)
                nc.vector.tensor_scalar_mul(neghalf[:, :], ab[:, :], -0.5)

                # x^T shard resident in SBUF as fp8 hi (+ lo residual for
                # the upper half of K), xg column groups.
                if mode == "pe":
                    xres = pe_init["xres"]
                    xlo = pe_init["xlo"]
                else:
                    xres = [
                        xres_pool.tile([P, ko, xw], FP8, tag=f"xres{g}",
                                       name=f"xres{g}")
                        for g in range(xg)
                    ]
                    xlo = [
                        xres_pool.tile([P, ko_lo, xw], FP8, tag=f"xlo{g}",
                                       name=f"xlo{g}")
                        for g in range(xg)
                    ]

                def load_x_group(g, casts=True):
                    for k8 in range(ko // 8):
                        st = stage.tile([P, 8, xw], F32, tag="xstage")
                        nc.sync.dma_start(
                            out=st[:, :, :],
                            in_=xt_d.ap()[:, k8 * 8:(k8 + 1) * 8,
                                          g * xw:(g + 1) * xw],
                        )
                        if not casts:
                            continue
                        hi = xres[g][:, k8 * 8:(k8 + 1) * 8, :]
                        nc.vector.tensor_copy(hi, st[:, :, :])
                        if k8 * 8 >= ko_lo:
                            # lo = x - hi for k-tiles 16..31 (fp8 out)
                            nc.vector.tensor_tensor(
                                xlo[g][:, k8 * 8 - ko_lo:(k8 + 1) * 8 - ko_lo, :],
                                st[:, :, :], hi,
                                mybir.AluOpType.subtract,
                            )

                if mode in ("xonly", "dmax"):
                    for g in range(xg):
                        load_x_group(g, casts=(mode == "xonly"))
                    wq0 = None

                def make_wq(n):
                    # Stream + ternarize one n-tile's W^T panel into ONE
                    # fp8 tile: matmuls carry a single wait per n-tile.
                    if mode == "pe":
                        return pe_init["wq"]
                    wq = wqp.tile([P, ko, N_TILE], FP8, tag="wq", name="wq")
                    half_ko = ko // 2
                    for h in range(2):
                        s2 = s2p.tile([P, half_ko, N_TILE], FP8, tag="s2",
                                      name="s2")
                        for c4 in range(half_ko // wchunk):
                            c = h * (half_ko // wchunk) + c4
                            st = wstage.tile([P, wchunk, N_TILE], F32,
                                             tag="wst")
                            nc.sync.dma_start(
                                out=st[:, :, :],
                                in_=wt_d.ap()[:, c * wchunk:(c + 1) * wchunk,
                                              n * N_TILE:(n + 1) * N_TILE],
                            )
                            qs = wq[:, c * wchunk:(c + 1) * wchunk, :]
                            nc.scalar.sign(qs, st[:, :, :], bias=half[:, :])
                            nc.scalar.sign(
                                s2[:, c4 * wchunk:(c4 + 1) * wchunk, :],
                                st[:, :, :], bias=neghalf[:, :])
                        # One DVE add per half-panel (fewer DVE ops).
                        hs = wq[:, h * half_ko:(h + 1) * half_ko, :]
                        nc.vector.tensor_tensor(
                            hs, hs, s2[:, :, :], mybir.AluOpType.add
                        )
                    return wq

                # Head order: first x group, first W panel, then the rest
                # of x — PE starts early instead of after the full x load.
                skip_x = ("pe", "xonly", "dmax")
                if mode not in skip_x:
                    load_x_group(0)
                wq0 = make_wq(0) if mode not in ("xonly", "dmax") else None

                mg = xw // P  # m-groups per xres tile

                wq_next = {0: wq0}
                for n in range(n_tiles if mode not in ("xonly", "dmax") else 0):
                    wq = wq_next.pop(n)
                    if mode == "dma":
                        if n + 1 < n_tiles:
                            wq_next[n + 1] = make_wq(n + 1)
                        continue

                    for m in range(m_sub):
                        g, col = divmod(m, mg)
                        cs = slice(col * P, (col + 1) * P)
                        ps = psum.tile([P, N_TILE], F32, tag="ps", name="ps")
                        for kg in range(ko // 2):
                            # hi pass: DoubleRow over k-tile pair (2kg, 2kg+1)
                            nc.tensor.matmul(
                                ps[:, :],
                                lhsT=xres[g][:, 2 * kg:2 * kg + 2, cs],
                                rhs=wq[:, 2 * kg:2 * kg + 2, :],
                                start=(kg == 0),
                                stop=False,
                                perf_mode=DR,
                            )
                        for kg in range(ko_lo // 2):
                            # lo pass: residual for k-tiles 16..31
                            nc.tensor.matmul(
                                ps[:, :],
                                lhsT=xlo[g][:, 2 * kg:2 * kg + 2, cs],
                                rhs=wq[:, ko_lo + 2 * kg:ko_lo + 2 * kg + 2, :],
                                start=False,
                                stop=(kg == ko_lo // 2 - 1),
                                perf_mode=DR,
                            )
                        ot = outs.tile([P, N_TILE], F32, tag="ot", name="ot")
                        # out = psum * (alpha/2): undoes the {-2,0,2}
                        # doubling and applies the alpha weight scale.
                        nc.scalar.mul(ot[:, :], ps[:, :], half[:, :])
                        # ACT's HWDGE ring, so output stores don't queue
                        # behind the input stream on the SP ring.
                        nc.scalar.dma_start(
                            out=out_d.ap()[:, m, n * N_TILE:(n + 1) * N_TILE],
                            in_=ot[:, :],
                        )
                        if n == 0 and mode not in skip_x and 0 <= m < xg - 1:
                            # JIT x loads: g1 after m=0, g2 after m=1, g3
                            # after m=2 — each lands just before the m-pair
                            # that reads it, ahead of wq1 on the DMA ring.
                            load_x_group(m + 1)
                        jit_m = (xg - 1) if n == 0 else 0
                        if m == jit_m and n + 1 < n_tiles:
                            # Software-pipeline the next panel's production
                            # HERE: its DVE adds land between this panel's
                            # evicts in DVE program order, so wq(n+1) is
                            # ready before panel n ends.
                            wq_next[n + 1] = make_wq(n + 1)

    nc.compile()
    return nc


_NC_CACHE = {}


def _get_nc():
    if "nc" not in _NC_CACHE:
        _NC_CACHE["nc"] = build()
    return _NC_CACHE["nc"]


def make_in_maps(x, W, alpha):
    x = np.ascontiguousarray(np.asarray(x, np.float32)).reshape(M_TOT, D_IN)
    W = np.ascontiguousarray(np.asarray(W, np.float32))
    a = np.full((1, 1), np.float32(np.asarray(alpha)), np.float32)
    # Per col-group: wt[p, k, n] = W[cg*N_SHARD + n, k*128 + p]
    wts = []
    for cg in range(CG):
        ws = W[cg * N_SHARD:(cg + 1) * N_SHARD]
        wts.append(np.ascontiguousarray(
            ws.reshape(N_SHARD, KO, P).transpose(2, 1, 0)))
    # Per row-group: xt[p, k, m] = xs[m, k*128 + p]
    xts = []
    for rg in range(RG):
        xs = x[rg * M_SHARD:(rg + 1) * M_SHARD]
        xts.append(np.ascontiguousarray(
            xs.reshape(M_SHARD, KO, P).transpose(2, 1, 0)))
    in_maps = []
    for c in range(N_CORES):
        rg, cg = divmod(c, CG)
        in_maps.append({"xt": xts[rg], "wt": wts[cg], "alpha": a})
    return in_maps


def gather_out(results):
    m_sub = M_SHARD // P
    full = np.empty((M_TOT, D_OUT), np.float32)
    for c in range(N_CORES):
        rg, cg = divmod(c, CG)
        o = results[c]["out"]  # [P, m_sub, N_SHARD]; row = mo*128 + p
        full[rg * M_SHARD:(rg + 1) * M_SHARD,
             cg * N_SHARD:(cg + 1) * N_SHARD] = (
            o.transpose(1, 0, 2).reshape(M_SHARD, N_SHARD))
    return full.reshape(4, 2048, D_OUT)


def kernel(x, W, alpha):
    nc = _get_nc()
    in_maps = make_in_maps(x, W, alpha)
    res = run_bass_kernel_spmd(nc, in_maps, core_ids=list(range(N_CORES)))
    return gather_out(res.results)


# revision 9
# speedup vs baseline: 1.0481x; 1.0481x over previous
"""BitNetLinear forward on 8 TRN2 NeuronCores — fp8 DoubleRow version.

out = x @ (alpha * clip(round(W/alpha), -1, 1))^T
  x [4, 2048, 4096] f32, W [4096, 4096] f32, alpha scalar f32.

Strategy: hybrid 4x2 tensor-parallel — 4 row-groups over the 8192 x-rows
x 2 column-groups over the 4096 out-features. Each core computes a
[2048, 2048] out tile from its x shard [2048, 4096] and W shard
[2048, 4096]. This halves the per-core W traffic AND the per-core
ternarization (ScalarE sign) work vs pure data-parallel, which is what
lets the fp8 PE stream run unstalled. No collectives. Host side only
reshapes/slices (layout); all arithmetic (ternary quantization + fp8
casts + matmul + alpha scaling) runs on device.

Device kernel (per core) — all-fp8 split-K with hi/lo error compensation:
  - W^T streamed in f32, ternarized on the fly to fp8e4 via
    T' = Sign(w + a/2) + Sign(w - a/2) in {-2, 0, 2} (exact in fp8).
  - x^T shard resident in SBUF as fp8e4 "hi" = fp8(x) for all K, plus a
    "lo" residual fp8(x - hi) for the upper half of K (k-tiles 16..31).
    Single-fp8 halves the matmul work vs bf16 (DoubleRow contracts 256/MM
    at the same 512-col stream rate); the hi+lo half restores accuracy
    there, yielding rel err ~1.66e-2 (< 2e-2) at 0.75x the bf16 PE time.
  - Per psum group: 16 hi DR-matmuls (K pairs 0..31) + 8 lo DR-matmuls
    (K pairs 16..31) accumulate into one PSUM bank; evicted through
    ScalarE with scale = alpha/2 (undoes the {-2,0,2} doubling and
    applies the alpha weight scale), DMA to out on ACT's HWDGE ring.
"""

import contextlib
import sys

if "/opt/trn_rl_repo" not in sys.path:
    sys.path.insert(0, "/opt/trn_rl_repo")

import numpy as np

import concourse.bass as bass  # noqa: F401
import concourse.mybir as mybir
import concourse.tile as tile
from concourse import bacc
from concourse.bass_utils import run_bass_kernel_spmd

P = 128
N_CORES = 8
D_IN = 4096  # contraction
D_OUT = 4096
M_TOT = 4 * 2048
RG = 4  # row groups (x-row parallel)
CG = 2  # col groups (out-feature parallel)
M_SHARD = M_TOT // RG  # 2048 rows per core
N_SHARD = D_OUT // CG  # 2048 out-features per core
KO = D_IN // P  # 32 k-tiles
KO_LO = KO // 2  # k-tiles 16..31 get the lo residual pass
N_TILE = 512

F32 = mybir.dt.float32
BF16 = mybir.dt.bfloat16
FP8 = mybir.dt.float8e4
DR = mybir.MatmulPerfMode.DoubleRow


def build(m_shard=M_SHARD, d_in=D_IN, d_out=N_SHARD, reps=1, mode="full",
          wstage_bufs=3, wchunk=4, outs_bufs=4):
    """mode: 'full' (real kernel), 'pe' (timing probe: no input DMA/quant),
    'dma' (W DMA + quant, no matmuls), 'dmax' (x DMA only)."""
    ko = d_in // P
    ko_lo = ko // 2
    n_tiles = d_out // N_TILE
    m_sub = m_shard // P
    xg = max(1, m_shard // 256)
    xw = m_shard // xg

    nc = bacc.Bacc("TRN2", target_bir_lowering=False, debug=False,
                   num_devices=N_CORES)
    xt_d = nc.declare_dram_parameter("xt", [P, ko, m_shard], F32, isOutput=False)
    wt_d = nc.declare_dram_parameter("wt", [P, ko, d_out], F32, isOutput=False)
    al_d = nc.declare_dram_parameter("alpha", [1, 1], F32, isOutput=False)
    out_d = nc.declare_dram_parameter("out", [P, m_sub, d_out], F32, isOutput=True)

    with tile.TileContext(nc) as tc:
        with (
            tc.tile_pool(name="const", bufs=1) as const,
            tc.tile_pool(name="xres", bufs=1) as xres_pool,
            tc.tile_pool(name="stage", bufs=2) as stage,
            tc.tile_pool(name="wstage", bufs=wstage_bufs) as wstage,
            tc.tile_pool(name="wq", bufs=n_tiles) as wqp,
            tc.tile_pool(name="s2", bufs=2) as s2p,
            tc.tile_pool(name="outs", bufs=outs_bufs) as outs,
            tc.tile_pool(name="psum", bufs=8, space="PSUM") as psum,
        ):
            pe_init = {}
            if mode == "pe":
                pe_init["xres"] = [
                    xres_pool.tile([P, ko, xw], FP8, tag=f"xres{g}",
                                   name=f"xres{g}")
                    for g in range(xg)
                ]
                pe_init["xlo"] = [
                    xres_pool.tile([P, ko_lo, xw], FP8, tag=f"xlo{g}",
                                   name=f"xlo{g}")
                    for g in range(xg)
                ]
                for g in range(xg):
                    nc.vector.memset(pe_init["xres"][g][:], 0.0)
                    nc.vector.memset(pe_init["xlo"][g][:], 0.0)
                pe_init["wq"] = wqp.tile([P, ko, N_TILE], FP8, tag="wq",
                                         name="wq_static")
                nc.vector.memset(pe_init["wq"][:], 0.0)

            rep_ctx = (
                tc.For_i(0, reps, 1) if reps > 1 else contextlib.nullcontext()
            )
            with rep_ctx:
                # alpha -> [1,1] -> broadcast to [128,1]; +a/2 and -a/2.
                a1 = const.tile([1, 1], F32)
                nc.sync.dma_start(out=a1[:, :], in_=al_d.ap()[:, :])
                ab = const.tile([P, 1], F32)
                nc.gpsimd.partition_broadcast(ab[:, :], a1[:, :])
                half = const.tile([P, 1], F32)
                nc.vector.tensor_scalar_mul(half[:, :], ab[:, :], 0.5)
                neghalf = const.tile([P, 1], F32)
                nc.vector.tensor_scalar_mul(neghalf[:, :], ab[:, :], -0.5)

                # x^T shard resident in SBUF as fp8 hi (+ lo residual for
                # the upper half of K), xg column groups.
                if mode == "pe":
                    xres = pe_init["xres"]
                    xlo = pe_init["xlo"]
                else:
                    xres = [
                        xres_pool.tile([P, ko, xw], FP8, tag=f"xres{g}",
                                       name=f"xres{g}")
                        for g in range(xg)
                    ]
                    xlo = [
                        xres_pool.tile([P, ko_lo, xw], FP8, tag=f"xlo{g}",
                                       name=f"xlo{g}")
                        for g in range(xg)
                    ]

                def load_x_group(g, casts=True):
                    for k4 in range(ko // 4):
                        st = stage.tile([P, 4, xw], F32, tag="xstage")
                        nc.sync.dma_start(
                            out=st[:, :, :],
                            in_=xt_d.ap()[:, k4 * 4:(k4 + 1) * 4,
                                          g * xw:(g + 1) * xw],
                        )
                        if not casts:
                            continue
                        hi = xres[g][:, k4 * 4:(k4 + 1) * 4, :]
                        nc.vector.tensor_copy(hi, st[:, :, :])
                        if k4 * 4 >= ko_lo:
                            # lo = x - hi for k-tiles 16..31 (fp8 out)
                            nc.vector.tensor_tensor(
                                xlo[g][:, k4 * 4 - ko_lo:(k4 + 1) * 4 - ko_lo, :],
                                st[:, :, :], hi,
                                mybir.AluOpType.subtract,
                            )

                if mode in ("xonly", "dmax"):
                    for g in range(xg):
                        load_x_group(g, casts=(mode == "xonly"))
                    wq0 = None

                def make_wq(n):
                    # Stream + ternarize one n-tile's W^T panel into ONE
                    # resident fp8 tile. Per-chunk sign/sign/add so the
                    # panel becomes usable k-chunk by k-chunk (short head,
                    # fine-grained PE gating via Tile semaphores).
                    if mode == "pe":
                        return pe_init["wq"]
                    wq = wqp.tile([P, ko, N_TILE], FP8, tag="wq", name="wq")
                    for c in range(ko // wchunk):
                        st = wstage.tile([P, wchunk, N_TILE], F32,
                                         tag="wst")
                        nc.sync.dma_start(
                            out=st[:, :, :],
                            in_=wt_d.ap()[:, c * wchunk:(c + 1) * wchunk,
                                          n * N_TILE:(n + 1) * N_TILE],
                        )
                        qs = wq[:, c * wchunk:(c + 1) * wchunk, :]
                        s2 = s2p.tile([P, wchunk, N_TILE], FP8, tag="s2",
                                      name="s2")
                        nc.scalar.sign(qs, st[:, :, :], bias=half[:, :])
                        nc.scalar.sign(s2[:, :, :], st[:, :, :],
                                       bias=neghalf[:, :])
                        nc.vector.tensor_tensor(
                            qs, qs, s2[:, :, :], mybir.AluOpType.add
                        )
                    return wq

                # Two-sweep schedule with all W panels resident as fp8:
                # sweep 0 covers m-tiles [0, m_sub/2) while the rest of x
                # and the remaining W panels stream in; sweep 1 covers
                # m-tiles [m_sub/2, m_sub) with zero input DMA left, so
                # the DMA front-log (x fully needed by first panel's end)
                # never stalls the PE.
                skip_x = ("pe", "xonly", "dmax")
                if mode not in skip_x:
                    load_x_group(0)
                wq_tiles = {}
                if mode not in ("xonly", "dmax"):
                    wq_tiles[0] = make_wq(0)
                if mode == "dma":
                    for n in range(1, n_tiles):
                        wq_tiles[n] = make_wq(n)

                mg = xw // P  # m-groups per xres tile
                mh = m_sub // 2  # m-tiles per sweep
                gh = mh // mg  # x groups per sweep
                # x groups for sweep 1, loaded during sweep 0 panels:
                # (panel, mi) -> group index
                late_g = {}
                if xg > gh:
                    slots = [(1, mh - 1), (2, mh - 1), (3, mh // 2 - 1),
                             (3, mh - 1)]
                    for i, g in enumerate(range(gh, xg)):
                        late_g[slots[i % len(slots)]] = g

                run_mms = mode not in ("xonly", "dmax", "dma")
                for half_i in range(2 if run_mms else 0):
                    for n in range(n_tiles):
                        wq = wq_tiles[n]
                        for mi in range(mh):
                            m = half_i * mh + mi
                            g, col = divmod(m, mg)
                            cs = slice(col * P, (col + 1) * P)
                            ps = psum.tile([P, N_TILE], F32, tag="ps",
                                           name="ps")
                            for kg in range(ko // 2):
                                # hi pass: DoubleRow over k-tile pair
                                nc.tensor.matmul(
                                    ps[:, :],
                                    lhsT=xres[g][:, 2 * kg:2 * kg + 2, cs],
                                    rhs=wq[:, 2 * kg:2 * kg + 2, :],
                                    start=(kg == 0),
                                    stop=False,
                                    perf_mode=DR,
                                )
                            for kg in range(ko_lo // 2):
                                # lo pass: residual for k-tiles 16..31
                                nc.tensor.matmul(
                                    ps[:, :],
                                    lhsT=xlo[g][:, 2 * kg:2 * kg + 2, cs],
                                    rhs=wq[:, ko_lo + 2 * kg:
                                           ko_lo + 2 * kg + 2, :],
                                    start=False,
                                    stop=(kg == ko_lo // 2 - 1),
                                    perf_mode=DR,
                                )
                            ot = outs.tile([P, N_TILE], F32, tag="ot",
                                           name="ot")
                            # out = psum * (alpha/2): undoes the {-2,0,2}
                            # doubling and applies the alpha weight scale.
                            nc.scalar.mul(ot[:, :], ps[:, :], half[:, :])
                            # ACT's HWDGE ring, so output stores don't
                            # queue behind the input stream on SP's ring.
                            nc.scalar.dma_start(
                                out=out_d.ap()[:, m,
                                               n * N_TILE:(n + 1) * N_TILE],
                                in_=ot[:, :],
                            )
                            if half_i == 0 and mode not in skip_x:
                                if (n == 0 and mi % mg == 0
                                        and mi // mg + 1 < gh):
                                    # JIT x for this sweep: g1.. just ahead
                                    # of the m-pair that reads them.
                                    load_x_group(mi // mg + 1)
                                if (n, mi) in late_g:
                                    load_x_group(late_g[(n, mi)])
                            if (half_i == 0 and mi == mh - 2
                                    and n + 1 < n_tiles
                                    and n + 1 not in wq_tiles):
                                # Software-pipeline the next panel's
                                # production between this panel's evicts.
                                wq_tiles[n + 1] = make_wq(n + 1)

    nc.compile()
    return nc


_NC_CACHE = {}


def _get_nc():
    if "nc" not in _NC_CACHE:
        _NC_CACHE["nc"] = build()
    return _NC_CACHE["nc"]


def make_in_maps(x, W, alpha):
    x = np.ascontiguousarray(np.asarray(x, np.float32)).reshape(M_TOT, D_IN)
    W = np.ascontiguousarray(np.asarray(W, np.float32))
    a = np.full((1, 1), np.float32(np.asarray(alpha)), np.float32)
    # Per col-group: wt[p, k, n] = W[cg*N_SHARD + n, k*128 + p]
    wts = []
    for cg in range(CG):
        ws = W[cg * N_SHARD:(cg + 1) * N_SHARD]
        wts.append(np.ascontiguousarray(
            ws.reshape(N_SHARD, KO, P).transpose(2, 1, 0)))
    # Per row-group: xt[p, k, m] = xs[m, k*128 + p]
    xts = []
    for rg in range(RG):
        xs = x[rg * M_SHARD:(rg + 1) * M_SHARD]
        xts.append(np.ascontiguousarray(
            xs.reshape(M_SHARD, KO, P).transpose(2, 1, 0)))
    in_maps = []
    for c in range(N_CORES):
        rg, cg = divmod(c, CG)
        in_maps.append({"xt": xts[rg], "wt": wts[cg], "alpha": a})
    return in_maps


def gather_out(results):
    m_sub = M_SHARD // P
    full = np.empty((M_TOT, D_OUT), np.float32)
    for c in range(N_CORES):
        rg, cg = divmod(c, CG)
        o = results[c]["out"]  # [P, m_sub, N_SHARD]; row = mo*128 + p
        full[rg * M_SHARD:(rg + 1) * M_SHARD,
             cg * N_SHARD:(cg + 1) * N_SHARD] = (
            o.transpose(1, 0, 2).reshape(M_SHARD, N_SHARD))
    return full.reshape(4, 2048, D_OUT)


def kernel(x, W, alpha):
    nc = _get_nc()
    in_maps = make_in_maps(x, W, alpha)
    res = run_bass_kernel_spmd(nc, in_maps, core_ids=list(range(N_CORES)))
    return gather_out(res.results)


# revision 11
# speedup vs baseline: 1.0924x; 1.0423x over previous
"""BitNetLinear forward on 8 TRN2 NeuronCores — fp8 DoubleRow version.

out = x @ (alpha * clip(round(W/alpha), -1, 1))^T
  x [4, 2048, 4096] f32, W [4096, 4096] f32, alpha scalar f32.

Strategy: hybrid 4x2 tensor-parallel — 4 row-groups over the 8192 x-rows
x 2 column-groups over the 4096 out-features. Each core computes a
[2048, 2048] out tile from its x shard [2048, 4096] and W shard
[2048, 4096]. This halves the per-core W traffic AND the per-core
ternarization (ScalarE sign) work vs pure data-parallel, which is what
lets the fp8 PE stream run unstalled. No collectives. Host side only
reshapes/slices (layout); all arithmetic (ternary quantization + fp8
casts + matmul + alpha scaling) runs on device.

Device kernel (per core) — all-fp8 split-K with hi/lo error compensation:
  - W^T streamed in f32, ternarized on the fly to fp8e4 via
    T' = Sign(w + a/2) + Sign(w - a/2) in {-2, 0, 2} (exact in fp8);
    all four quantized W panels stay resident in SBUF (8.4 MB).
  - x^T shard resident in SBUF as fp8e4 "hi" = fp8(x) for all K, plus a
    "lo" residual fp8(x - hi) for the upper half of K (k-tiles 16..31).
    Single-fp8 halves the matmul work vs bf16 (DoubleRow contracts 256/MM
    at the same 512-col stream rate); the hi+lo half restores accuracy
    there, yielding rel err ~1.66e-2 (< 2e-2) at 0.75x the bf16 PE time.
  - Per psum group: 16 hi DR-matmuls (K pairs 0..31) + 8 lo DR-matmuls
    (K pairs 16..31) accumulate into one PSUM bank; evicted through
    ScalarE with scale = alpha/2 (undoes the {-2,0,2} doubling and
    applies the alpha weight scale), DMA to out on ACT's HWDGE ring.
  - Wavefront schedule: the input DMA front (x 33.6 MB f32 + W 33.6 MB
    f32 at ~358 GB/s) cannot keep a naive panel-major loop fed early on.
    Instead, psum groups are emitted in arrival order: after each x
    group (4.2 MB) or W panel (8.4 MB) lands, every newly enabled
    (panel x group) cell runs, so enabled PE work grows with the product
    of loaded inputs and the PE never starves for long. Per-k-chunk
    quantization (DMA chunk -> 2 signs -> DVE add) lets the PE start
    ~15 us into the kernel on the first panel's first chunks.
"""

import contextlib
import sys

if "/opt/trn_rl_repo" not in sys.path:
    sys.path.insert(0, "/opt/trn_rl_repo")

import numpy as np

import concourse.bass as bass  # noqa: F401
import concourse.mybir as mybir
import concourse.tile as tile
from concourse import bacc
from concourse.bass_utils import run_bass_kernel_spmd

P = 128
N_CORES = 8
D_IN = 4096  # contraction
D_OUT = 4096
M_TOT = 4 * 2048
RG = 4  # row groups (x-row parallel)
CG = 2  # col groups (out-feature parallel)
M_SHARD = M_TOT // RG  # 2048 rows per core
N_SHARD = D_OUT // CG  # 2048 out-features per core
KO = D_IN // P  # 32 k-tiles
KO_LO = KO // 2  # k-tiles 16..31 get the lo residual pass
N_TILE = 512

F32 = mybir.dt.float32
BF16 = mybir.dt.bfloat16
FP8 = mybir.dt.float8e4
DR = mybir.MatmulPerfMode.DoubleRow


def build(m_shard=M_SHARD, d_in=D_IN, d_out=N_SHARD, reps=1, mode="full",
          wstage_bufs=3, wchunk=4, outs_bufs=4):
    """mode: 'full' (real kernel), 'pe' (timing probe: no input DMA/quant),
    'dma' (W DMA + quant, no matmuls), 'dmax' (x DMA only)."""
    ko = d_in // P
    ko_lo = ko // 2
    n_tiles = d_out // N_TILE
    m_sub = m_shard // P
    xg = max(1, m_shard // 256)
    xw = m_shard // xg

    nc = bacc.Bacc("TRN2", target_bir_lowering=False, debug=False,
                   num_devices=N_CORES)
    xt_d = nc.declare_dram_parameter("xt", [P, ko, m_shard], F32, isOutput=False)
    wt_d = nc.declare_dram_parameter("wt", [P, ko, d_out], F32, isOutput=False)
    al_d = nc.declare_dram_parameter("alpha", [1, 1], F32, isOutput=False)
    out_d = nc.declare_dram_parameter("out", [P, m_sub, d_out], F32, isOutput=True)

    with tile.TileContext(nc) as tc:
        with (
            tc.tile_pool(name="const", bufs=1) as const,
            tc.tile_pool(name="xres", bufs=1) as xres_pool,
            tc.tile_pool(name="stage", bufs=2) as stage,
            tc.tile_pool(name="wstage", bufs=wstage_bufs) as wstage,
            tc.tile_pool(name="wq", bufs=n_tiles) as wqp,
            tc.tile_pool(name="s2", bufs=2) as s2p,
            tc.tile_pool(name="outs", bufs=outs_bufs) as outs,
            tc.tile_pool(name="psum", bufs=8, space="PSUM") as psum,
        ):
            pe_init = {}
            if mode == "pe":
                pe_init["xres"] = [
                    xres_pool.tile([P, ko, xw], FP8, tag=f"xres{g}",
                                   name=f"xres{g}")
                    for g in range(xg)
                ]
                pe_init["xlo"] = [
                    xres_pool.tile([P, ko_lo, xw], FP8, tag=f"xlo{g}",
                                   name=f"xlo{g}")
                    for g in range(xg)
                ]
                for g in range(xg):
                    nc.vector.memset(pe_init["xres"][g][:], 0.0)
                    nc.vector.memset(pe_init["xlo"][g][:], 0.0)
                pe_init["wq"] = wqp.tile([P, ko, N_TILE], FP8, tag="wq",
                                         name="wq_static")
                nc.vector.memset(pe_init["wq"][:], 0.0)

            rep_ctx = (
                tc.For_i(0, reps, 1) if reps > 1 else contextlib.nullcontext()
            )
            with rep_ctx:
                # alpha -> [1,1] -> broadcast to [128,1]; +a/2 and -a/2.
                a1 = const.tile([1, 1], F32)
                nc.sync.dma_start(out=a1[:, :], in_=al_d.ap()[:, :])
                ab = const.tile([P, 1], F32)
                nc.gpsimd.partition_broadcast(ab[:, :], a1[:, :])
                half = const.tile([P, 1], F32)
                nc.vector.tensor_scalar_mul(half[:, :], ab[:, :], 0.5)
                neghalf = const.tile([P, 1], F32)
                nc.vector.tensor_scalar_mul(neghalf[:, :], ab[:, :], -0.5)

                # x^T shard resident in SBUF as fp8 hi (+ lo residual for
                # the upper half of K), xg column groups.
                if mode == "pe":
                    xres = pe_init["xres"]
                    xlo = pe_init["xlo"]
                else:
                    xres = [
                        xres_pool.tile([P, ko, xw], FP8, tag=f"xres{g}",
                                       name=f"xres{g}")
                        for g in range(xg)
                    ]
                    xlo = [
                        xres_pool.tile([P, ko_lo, xw], FP8, tag=f"xlo{g}",
                                       name=f"xlo{g}")
                        for g in range(xg)
                    ]

                def load_x_group(g, casts=True):
                    for k4 in range(ko // 4):
                        st = stage.tile([P, 4, xw], F32, tag="xstage")
                        nc.sync.dma_start(
                            out=st[:, :, :],
                            in_=xt_d.ap()[:, k4 * 4:(k4 + 1) * 4,
                                          g * xw:(g + 1) * xw],
                        )
                        if not casts:
                            continue
                        hi = xres[g][:, k4 * 4:(k4 + 1) * 4, :]
                        nc.vector.tensor_copy(hi, st[:, :, :])
                        if k4 * 4 >= ko_lo:
                            # lo = x - hi for k-tiles 16..31 (fp8 out)
                            nc.vector.tensor_tensor(
                                xlo[g][:, k4 * 4 - ko_lo:(k4 + 1) * 4 - ko_lo, :],
                                st[:, :, :], hi,
                                mybir.AluOpType.subtract,
                            )

                if mode in ("xonly", "dmax"):
                    for g in range(xg):
                        load_x_group(g, casts=(mode == "xonly"))
                    wq0 = None

                def make_wq(n):
                    # Stream + ternarize one n-tile's W^T panel into ONE
                    # resident fp8 tile. Per-chunk sign/sign/add so the
                    # panel becomes usable k-chunk by k-chunk (short head,
                    # fine-grained PE gating via Tile semaphores).
                    if mode == "pe":
                        return pe_init["wq"]
                    wq = wqp.tile([P, ko, N_TILE], FP8, tag="wq", name="wq")
                    for c in range(ko // wchunk):
                        st = wstage.tile([P, wchunk, N_TILE], F32,
                                         tag="wst")
                        nc.sync.dma_start(
                            out=st[:, :, :],
                            in_=wt_d.ap()[:, c * wchunk:(c + 1) * wchunk,
                                          n * N_TILE:(n + 1) * N_TILE],
                        )
                        qs = wq[:, c * wchunk:(c + 1) * wchunk, :]
                        s2 = s2p.tile([P, wchunk, N_TILE], FP8, tag="s2",
                                      name="s2")
                        nc.scalar.sign(qs, st[:, :, :], bias=half[:, :])
                        nc.scalar.sign(s2[:, :, :], st[:, :, :],
                                       bias=neghalf[:, :])
                        nc.vector.tensor_tensor(
                            qs, qs, s2[:, :, :], mybir.AluOpType.add
                        )
                    return wq

                mg = xw // P  # m-groups per xres tile
                wq_tiles = {}

                def emit_cell(n, g):
                    # One cell = panel n x x-group g = mg psum groups.
                    wq = wq_tiles[n]
                    for col in range(mg):
                        m = g * mg + col
                        cs = slice(col * P, (col + 1) * P)
                        ps = psum.tile([P, N_TILE], F32, tag="ps",
                                       name="ps")
                        for kg in range(ko // 2):
                            # hi pass: DoubleRow over k-tile pair
                            nc.tensor.matmul(
                                ps[:, :],
                                lhsT=xres[g][:, 2 * kg:2 * kg + 2, cs],
                                rhs=wq[:, 2 * kg:2 * kg + 2, :],
                                start=(kg == 0),
                                stop=False,
                                perf_mode=DR,
                            )
                        for kg in range(ko_lo // 2):
                            # lo pass: residual for k-tiles 16..31
                            nc.tensor.matmul(
                                ps[:, :],
                                lhsT=xlo[g][:, 2 * kg:2 * kg + 2, cs],
                                rhs=wq[:, ko_lo + 2 * kg:
                                       ko_lo + 2 * kg + 2, :],
                                start=False,
                                stop=(kg == ko_lo // 2 - 1),
                                perf_mode=DR,
                            )
                        ot = outs.tile([P, N_TILE], F32, tag="ot",
                                       name="ot")
                        # out = psum * (alpha/2): undoes the {-2,0,2}
                        # doubling and applies the alpha weight scale.
                        nc.scalar.mul(ot[:, :], ps[:, :], half[:, :])
                        # ACT's HWDGE ring, so output stores don't
                        # queue behind the input stream on SP's ring.
                        nc.scalar.dma_start(
                            out=out_d.ap()[:, m,
                                           n * N_TILE:(n + 1) * N_TILE],
                            in_=ot[:, :],
                        )

                # Wavefront schedule: all W panels stay resident as fp8
                # (8.4 MB total), x groups stream in; after every arrival
                # (x group or W panel) emit all newly-enabled cells. The
                # enabled PE work grows with loaded_x * loaded_W, so the
                # oversubscribed DMA front never starves the PE for long.
                if mode == "pe":
                    for n in range(n_tiles):
                        wq_tiles[n] = pe_init["wq"]
                    for n in range(n_tiles):
                        for g in range(xg):
                            emit_cell(n, g)
                elif mode == "dma":
                    for n in range(n_tiles):
                        wq_tiles[n] = make_wq(n)
                elif mode in ("xonly", "dmax"):
                    for g in range(xg):
                        load_x_group(g, casts=(mode == "xonly"))
                else:
                    if n_tiles == 4 and xg == 8:
                        # Hand-tuned arrival order (x group = 4.2 MB,
                        # W panel = 8.4 MB on the same DMA ring).
                        events = [("x", 0), ("w", 0), ("w", 1), ("x", 1),
                                  ("x", 2), ("w", 2), ("x", 3), ("w", 3)]
                        events += [("x", i) for i in range(4, xg)]
                    else:
                        events = [("x", 0), ("w", 0)]
                        k = 1
                        while k < max(n_tiles, xg):
                            if k < n_tiles:
                                events.append(("w", k))
                            if k < xg:
                                events.append(("x", k))
                            k += 1
                    loaded_g, loaded_w = [], []
                    for kind, idx in events:
                        if kind == "x":
                            load_x_group(idx)
                            loaded_g.append(idx)
                            for n in loaded_w:
                                emit_cell(n, idx)
                        else:
                            wq_tiles[idx] = make_wq(idx)
                            loaded_w.append(idx)
                            for g in loaded_g:
                                emit_cell(idx, g)

    nc.compile()
    return nc


_NC_CACHE = {}


def _get_nc():
    if "nc" not in _NC_CACHE:
        _NC_CACHE["nc"] = build()
    return _NC_CACHE["nc"]


def make_in_maps(x, W, alpha):
    x = np.ascontiguousarray(np.asarray(x, np.float32)).reshape(M_TOT, D_IN)
    W = np.ascontiguousarray(np.asarray(W, np.float32))
    a = np.full((1, 1), np.float32(np.asarray(alpha)), np.float32)
    # Per col-group: wt[p, k, n] = W[cg*N_SHARD + n, k*128 + p]
    wts = []
    for cg in range(CG):
        ws = W[cg * N_SHARD:(cg + 1) * N_SHARD]
        wts.append(np.ascontiguousarray(
            ws.reshape(N_SHARD, KO, P).transpose(2, 1, 0)))
    # Per row-group: xt[p, k, m] = xs[m, k*128 + p]
    xts = []
    for rg in range(RG):
        xs = x[rg * M_SHARD:(rg + 1) * M_SHARD]
        xts.append(np.ascontiguousarray(
            xs.reshape(M_SHARD, KO, P).transpose(2, 1, 0)))
    in_maps = []
    for c in range(N_CORES):
        rg, cg = divmod(c, CG)
        in_maps.append({"xt": xts[rg], "wt": wts[cg], "alpha": a})
    return in_maps


def gather_out(results):
    m_sub = M_SHARD // P
    full = np.empty((M_TOT, D_OUT), np.float32)
    for c in range(N_CORES):
        rg, cg = divmod(c, CG)
        o = results[c]["out"]  # [P, m_sub, N_SHARD]; row = mo*128 + p
        full[rg * M_SHARD:(rg + 1) * M_SHARD,
             cg * N_SHARD:(cg + 1) * N_SHARD] = (
            o.transpose(1, 0, 2).reshape(M_SHARD, N_SHARD))
    return full.reshape(4, 2048, D_OUT)


def kernel(x, W, alpha):
    nc = _get_nc()
    in_maps = make_in_maps(x, W, alpha)
    res = run_bass_kernel_spmd(nc, in_maps, core_ids=list(range(N_CORES)))
    return gather_out(res.results)


# revision 14
# speedup vs baseline: 1.1197x; 1.0250x over previous
"""BitNetLinear forward on 8 TRN2 NeuronCores — fp8 DoubleRow version.

out = x @ (alpha * clip(round(W/alpha), -1, 1))^T
  x [4, 2048, 4096] f32, W [4096, 4096] f32, alpha scalar f32.

Strategy: hybrid 4x2 tensor-parallel — 4 row-groups over the 8192 x-rows
x 2 column-groups over the 4096 out-features. Each core computes a
[2048, 2048] out tile from its x shard [2048, 4096] and W shard
[2048, 4096]. This halves the per-core W traffic AND the per-core
ternarization (ScalarE sign) work vs pure data-parallel, which is what
lets the fp8 PE stream run unstalled. No collectives. Host side only
reshapes/slices (layout); all arithmetic (ternary quantization + fp8
casts + matmul + alpha scaling) runs on device.

Device kernel (per core) — all-fp8 split-K with hi/lo error compensation:
  - W^T streamed in f32, ternarized on the fly to fp8e4 via
    T' = Sign(w + a/2) + Sign(w - a/2) in {-2, 0, 2} (exact in fp8);
    all four quantized W panels stay resident in SBUF (8.4 MB).
  - x^T shard resident in SBUF as fp8e4 "hi" = fp8(x) for all K, plus a
    "lo" residual fp8(x - hi) for k-tiles 18..31. Single-fp8 halves the
    matmul work vs bf16 (DoubleRow contracts 256/MM at the same 512-col
    stream rate); the hi+lo range restores accuracy there, yielding
    rel err ~1.76e-2 (< 2e-2) at ~0.72x the bf16 PE time.
  - Per psum group: 16 hi DR-matmuls (k-tile pairs 0..31) + 7 lo
    DR-matmuls (pairs 18..31) accumulate into one PSUM bank; evicted via
    ScalarE with scale = alpha/2 (undoes the {-2,0,2} doubling and
    applies the alpha weight scale), DMA to out on ACT's HWDGE ring.
  - Wavefront schedule: the input DMA front (x 33.6 MB f32 + W 33.6 MB
    f32 at ~358 GB/s) cannot keep a naive panel-major loop fed early on.
    Instead, psum groups are emitted in arrival order: after each x
    group (4.2 MB) or W panel (8.4 MB) lands, every newly enabled
    (panel x group) cell runs, so enabled PE work grows with the product
    of loaded inputs and the PE never starves for long. Per-k-chunk
    quantization (DMA chunk -> 2 signs -> DVE add) lets the PE start
    ~15 us into the kernel on the first panel's first chunks.
"""

import contextlib
import sys

if "/opt/trn_rl_repo" not in sys.path:
    sys.path.insert(0, "/opt/trn_rl_repo")

import numpy as np

import concourse.bass as bass  # noqa: F401
import concourse.mybir as mybir
import concourse.tile as tile
from concourse import bacc
from concourse.bass_utils import run_bass_kernel_spmd

P = 128
N_CORES = 8
D_IN = 4096  # contraction
D_OUT = 4096
M_TOT = 4 * 2048
RG = 4  # row groups (x-row parallel)
CG = 2  # col groups (out-feature parallel)
M_SHARD = M_TOT // RG  # 2048 rows per core
N_SHARD = D_OUT // CG  # 2048 out-features per core
KO = D_IN // P  # 32 k-tiles
KO_LO_START = 18  # k-tiles 18..31 get the lo residual pass (rel err
KO_LO = KO - KO_LO_START  # ~1.76e-2 vs 1.66e-2 at 16, saves 1 DR MM/group)
N_TILE = 512

F32 = mybir.dt.float32
BF16 = mybir.dt.bfloat16
FP8 = mybir.dt.float8e4
DR = mybir.MatmulPerfMode.DoubleRow


def build(m_shard=M_SHARD, d_in=D_IN, d_out=N_SHARD, reps=1, mode="full",
          wstage_bufs=3, wchunk=4, outs_bufs=4):
    """mode: 'full' (real kernel), 'pe' (timing probe: no input DMA/quant),
    'dma' (W DMA + quant, no matmuls), 'dmax' (x DMA only)."""
    ko = d_in // P
    ko_lo_start = (ko * KO_LO_START) // KO
    ko_lo = ko - ko_lo_start
    n_tiles = d_out // N_TILE
    m_sub = m_shard // P
    xg = max(1, m_shard // 256)
    xw = m_shard // xg

    nc = bacc.Bacc("TRN2", target_bir_lowering=False, debug=False,
                   num_devices=N_CORES)
    xt_d = nc.declare_dram_parameter("xt", [P, ko, m_shard], F32, isOutput=False)
    wt_d = nc.declare_dram_parameter("wt", [P, ko, d_out], F32, isOutput=False)
    al_d = nc.declare_dram_parameter("alpha", [1, 1], F32, isOutput=False)
    out_d = nc.declare_dram_parameter("out", [P, m_sub, d_out], F32, isOutput=True)

    with tile.TileContext(nc) as tc:
        with (
            tc.tile_pool(name="const", bufs=1) as const,
            tc.tile_pool(name="xres", bufs=1) as xres_pool,
            tc.tile_pool(name="stage", bufs=2) as stage,
            tc.tile_pool(name="wstage", bufs=wstage_bufs) as wstage,
            tc.tile_pool(name="wq", bufs=n_tiles) as wqp,
            tc.tile_pool(name="s2", bufs=2) as s2p,
            tc.tile_pool(name="outs", bufs=outs_bufs) as outs,
            tc.tile_pool(name="psum", bufs=8, space="PSUM") as psum,
        ):
            pe_init = {}
            if mode == "pe":
                pe_init["xres"] = [
                    xres_pool.tile([P, ko, xw], FP8, tag=f"xres{g}",
                                   name=f"xres{g}")
                    for g in range(xg)
                ]
                pe_init["xlo"] = [
                    xres_pool.tile([P, ko_lo, xw], FP8, tag=f"xlo{g}",
                                   name=f"xlo{g}")
                    for g in range(xg)
                ]
                for g in range(xg):
                    nc.vector.memset(pe_init["xres"][g][:], 0.0)
                    nc.vector.memset(pe_init["xlo"][g][:], 0.0)
                pe_init["wq"] = wqp.tile([P, ko, N_TILE], FP8, tag="wq",
                                         name="wq_static")
                nc.vector.memset(pe_init["wq"][:], 0.0)

            rep_ctx = (
                tc.For_i(0, reps, 1) if reps > 1 else contextlib.nullcontext()
            )
            with rep_ctx:
                # alpha -> [1,1] -> broadcast to [128,1]; +a/2 and -a/2.
                a1 = const.tile([1, 1], F32)
                nc.sync.dma_start(out=a1[:, :], in_=al_d.ap()[:, :])
                ab = const.tile([P, 1], F32)
                nc.gpsimd.partition_broadcast(ab[:, :], a1[:, :])
                half = const.tile([P, 1], F32)
                nc.vector.tensor_scalar_mul(half[:, :], ab[:, :], 0.5)
                neghalf = const.tile([P, 1], F32)
                nc.vector.tensor_scalar_mul(neghalf[:, :], ab[:, :], -0.5)

                # x^T shard resident in SBUF as fp8 hi (+ lo residual for
                # the upper half of K), xg column groups.
                if mode == "pe":
                    xres = pe_init["xres"]
                    xlo = pe_init["xlo"]
                else:
                    xres = [
                        xres_pool.tile([P, ko, xw], FP8, tag=f"xres{g}",
                                       name=f"xres{g}")
                        for g in range(xg)
                    ]
                    xlo = [
                        xres_pool.tile([P, ko_lo, xw], FP8, tag=f"xlo{g}",
                                       name=f"xlo{g}")
                        for g in range(xg)
                    ]

                def load_x_group(g, casts=True):
                    for k4 in range(ko // 4):
                        st = stage.tile([P, 4, xw], F32, tag="xstage")
                        nc.sync.dma_start(
                            out=st[:, :, :],
                            in_=xt_d.ap()[:, k4 * 4:(k4 + 1) * 4,
                                          g * xw:(g + 1) * xw],
                        )
                        if not casts:
                            continue
                        hi = xres[g][:, k4 * 4:(k4 + 1) * 4, :]
                        nc.vector.tensor_copy(hi, st[:, :, :])
                        s0 = max(k4 * 4, ko_lo_start)
                        if s0 < (k4 + 1) * 4:
                            # lo = x - hi for k-tiles >= ko_lo_start
                            nc.vector.tensor_tensor(
                                xlo[g][:, s0 - ko_lo_start:
                                       (k4 + 1) * 4 - ko_lo_start, :],
                                st[:, s0 - k4 * 4:, :],
                                xres[g][:, s0:(k4 + 1) * 4, :],
                                mybir.AluOpType.subtract,
                            )

                if mode in ("xonly", "dmax"):
                    for g in range(xg):
                        load_x_group(g, casts=(mode == "xonly"))
                    wq0 = None

                def make_wq(n):
                    # Stream + ternarize one n-tile's W^T panel into ONE
                    # resident fp8 tile. Per-chunk sign/sign/add so the
                    # panel becomes usable k-chunk by k-chunk (short head,
                    # fine-grained PE gating via Tile semaphores).
                    if mode == "pe":
                        return pe_init["wq"]
                    wq = wqp.tile([P, ko, N_TILE], FP8, tag="wq", name="wq")
                    for c in range(ko // wchunk):
                        st = wstage.tile([P, wchunk, N_TILE], F32,
                                         tag="wst")
                        nc.sync.dma_start(
                            out=st[:, :, :],
                            in_=wt_d.ap()[:, c * wchunk:(c + 1) * wchunk,
                                          n * N_TILE:(n + 1) * N_TILE],
                        )
                        qs = wq[:, c * wchunk:(c + 1) * wchunk, :]
                        s2 = s2p.tile([P, wchunk, N_TILE], FP8, tag="s2",
                                      name="s2")
                        nc.scalar.sign(qs, st[:, :, :], bias=half[:, :])
                        nc.scalar.sign(s2[:, :, :], st[:, :, :],
                                       bias=neghalf[:, :])
                        nc.vector.tensor_tensor(
                            qs, qs, s2[:, :, :], mybir.AluOpType.add
                        )
                    return wq

                mg = xw // P  # m-groups per xres tile
                wq_tiles = {}

                def emit_cell(n, g):
                    # One cell = panel n x x-group g = mg psum groups.
                    wq = wq_tiles[n]
                    for col in range(mg):
                        m = g * mg + col
                        cs = slice(col * P, (col + 1) * P)
                        ps = psum.tile([P, N_TILE], F32, tag="ps",
                                       name="ps")
                        for kg in range(ko // 2):
                            # hi pass: DoubleRow over k-tile pair
                            nc.tensor.matmul(
                                ps[:, :],
                                lhsT=xres[g][:, 2 * kg:2 * kg + 2, cs],
                                rhs=wq[:, 2 * kg:2 * kg + 2, :],
                                start=(kg == 0),
                                stop=False,
                                perf_mode=DR,
                            )
                        for kg in range(ko_lo // 2):
                            # lo pass: residual for k-tiles >= ko_lo_start
                            nc.tensor.matmul(
                                ps[:, :],
                                lhsT=xlo[g][:, 2 * kg:2 * kg + 2, cs],
                                rhs=wq[:, ko_lo_start + 2 * kg:
                                       ko_lo_start + 2 * kg + 2, :],
                                start=False,
                                stop=(kg == ko_lo // 2 - 1),
                                perf_mode=DR,
                            )
                        ot = outs.tile([P, N_TILE], F32, tag="ot",
                                       name="ot")
                        # out = psum * (alpha/2): undoes the {-2,0,2}
                        # doubling and applies the alpha weight scale.
                        nc.scalar.mul(ot[:, :], ps[:, :], half[:, :])
                        # ACT's HWDGE ring, so output stores don't
                        # queue behind the input stream on SP's ring.
                        nc.scalar.dma_start(
                            out=out_d.ap()[:, m,
                                           n * N_TILE:(n + 1) * N_TILE],
                            in_=ot[:, :],
                        )

                # Wavefront schedule: all W panels stay resident as fp8
                # (8.4 MB total), x groups stream in; after every arrival
                # (x group or W panel) emit all newly-enabled cells. The
                # enabled PE work grows with loaded_x * loaded_W, so the
                # oversubscribed DMA front never starves the PE for long.
                if mode == "pe":
                    for n in range(n_tiles):
                        wq_tiles[n] = pe_init["wq"]
                    for n in range(n_tiles):
                        for g in range(xg):
                            emit_cell(n, g)
                elif mode == "dma":
                    for n in range(n_tiles):
                        wq_tiles[n] = make_wq(n)
                elif mode in ("xonly", "dmax"):
                    for g in range(xg):
                        load_x_group(g, casts=(mode == "xonly"))
                else:
                    if n_tiles == 4 and xg == 8:
                        # Hand-tuned arrival order (x group = 4.2 MB,
                        # W panel = 8.4 MB on the same DMA ring).
                        events = [("x", 0), ("w", 0), ("w", 1), ("x", 1),
                                  ("x", 2), ("w", 2), ("x", 3), ("w", 3)]
                        events += [("x", i) for i in range(4, xg)]
                    else:
                        events = [("x", 0), ("w", 0)]
                        k = 1
                        while k < max(n_tiles, xg):
                            if k < n_tiles:
                                events.append(("w", k))
                            if k < xg:
                                events.append(("x", k))
                            k += 1
                    loaded_g, loaded_w = [], []
                    for kind, idx in events:
                        if kind == "x":
                            load_x_group(idx)
                            loaded_g.append(idx)
                            for n in loaded_w:
                                emit_cell(n, idx)
                        else:
                            wq_tiles[idx] = make_wq(idx)
                            loaded_w.append(idx)
                            for g in loaded_g:
                                emit_cell(idx, g)

    nc.compile()
    return nc


_NC_CACHE = {}


def _get_nc():
    if "nc" not in _NC_CACHE:
        _NC_CACHE["nc"] = build()
    return _NC_CACHE["nc"]


def make_in_maps(x, W, alpha):
    x = np.ascontiguousarray(np.asarray(x, np.float32)).reshape(M_TOT, D_IN)
    W = np.ascontiguousarray(np.asarray(W, np.float32))
    a = np.full((1, 1), np.float32(np.asarray(alpha)), np.float32)
    # Per col-group: wt[p, k, n] = W[cg*N_SHARD + n, k*128 + p]
    wts = []
    for cg in range(CG):
        ws = W[cg * N_SHARD:(cg + 1) * N_SHARD]
        wts.append(np.ascontiguousarray(
            ws.reshape(N_SHARD, KO, P).transpose(2, 1, 0)))
    # Per row-group: xt[p, k, m] = xs[m, k*128 + p]
    xts = []
    for rg in range(RG):
        xs = x[rg * M_SHARD:(rg + 1) * M_SHARD]
        xts.append(np.ascontiguousarray(
            xs.reshape(M_SHARD, KO, P).transpose(2, 1, 0)))
    in_maps = []
    for c in range(N_CORES):
        rg, cg = divmod(c, CG)
        in_maps.append({"xt": xts[rg], "wt": wts[cg], "alpha": a})
    return in_maps


def gather_out(results):
    m_sub = M_SHARD // P
    full = np.empty((M_TOT, D_OUT), np.float32)
    for c in range(N_CORES):
        rg, cg = divmod(c, CG)
        o = results[c]["out"]  # [P, m_sub, N_SHARD]; row = mo*128 + p
        full[rg * M_SHARD:(rg + 1) * M_SHARD,
             cg * N_SHARD:(cg + 1) * N_SHARD] = (
            o.transpose(1, 0, 2).reshape(M_SHARD, N_SHARD))
    return full.reshape(4, 2048, D_OUT)


def kernel(x, W, alpha):
    nc = _get_nc()
    in_maps = make_in_maps(x, W, alpha)
    res = run_bass_kernel_spmd(nc, in_maps, core_ids=list(range(N_CORES)))
    return gather_out(res.results)


# revision 17
# speedup vs baseline: 1.1224x; 1.0024x over previous
"""BitNetLinear forward on 8 TRN2 NeuronCores — fp8 DoubleRow version.

out = x @ (alpha * clip(round(W/alpha), -1, 1))^T
  x [4, 2048, 4096] f32, W [4096, 4096] f32, alpha scalar f32.

Strategy: hybrid 4x2 tensor-parallel — 4 row-groups over the 8192 x-rows
x 2 column-groups over the 4096 out-features. Each core computes a
[2048, 2048] out tile from its x shard [2048, 4096] and W shard
[2048, 4096]. This halves the per-core W traffic AND the per-core
ternarization (ScalarE sign) work vs pure data-parallel, which is what
lets the fp8 PE stream run unstalled. No collectives. Host side only
reshapes/slices (layout); all arithmetic (ternary quantization + fp8
casts + matmul + alpha scaling) runs on device.

Device kernel (per core) — all-fp8 split-K with hi/lo error compensation:
  - W^T streamed in f32, ternarized on the fly to fp8e4 via
    T' = Sign(w + a/2) + Sign(w - a/2) in {-2, 0, 2} (exact in fp8);
    all four quantized W panels stay resident in SBUF (8.4 MB).
  - x^T shard resident in SBUF as fp8e4 "hi" = fp8(x) for all K, plus a
    "lo" residual fp8(x - hi) for k-tiles 18..31. Single-fp8 halves the
    matmul work vs bf16 (DoubleRow contracts 256/MM at the same 512-col
    stream rate); the hi+lo range restores accuracy there, yielding
    rel err ~1.76e-2 (< 2e-2) at ~0.72x the bf16 PE time.
  - Per psum group: 16 hi DR-matmuls (k-tile pairs 0..31) + 7 lo
    DR-matmuls (pairs 18..31) accumulate into one PSUM bank; evicted via
    ScalarE with scale = alpha/2 (undoes the {-2,0,2} doubling and
    applies the alpha weight scale), DMA to out on ACT's HWDGE ring.
  - Wavefront schedule: the input DMA front (x 33.6 MB f32 + W 33.6 MB
    f32 at ~358 GB/s) cannot keep a naive panel-major loop fed early on.
    Instead, psum groups are emitted in arrival order: after each x
    group (4.2 MB) or W panel (8.4 MB) lands, every newly enabled
    (panel x group) cell runs, so enabled PE work grows with the product
    of loaded inputs and the PE never starves for long. Per-k-chunk
    quantization (DMA chunk -> 2 signs -> DVE add) lets the PE start
    ~15 us into the kernel on the first panel's first chunks.
"""

import contextlib
import sys

if "/opt/trn_rl_repo" not in sys.path:
    sys.path.insert(0, "/opt/trn_rl_repo")

import numpy as np

import concourse.bass as bass  # noqa: F401
import concourse.mybir as mybir
import concourse.tile as tile
from concourse import bacc
from concourse.bass_utils import run_bass_kernel_spmd

P = 128
N_CORES = 8
D_IN = 4096  # contraction
D_OUT = 4096
M_TOT = 4 * 2048
RG = 4  # row groups (x-row parallel)
CG = 2  # col groups (out-feature parallel)
M_SHARD = M_TOT // RG  # 2048 rows per core
N_SHARD = D_OUT // CG  # 2048 out-features per core
KO = D_IN // P  # 32 k-tiles
KO_LO_START = 18  # k-tiles 18..31 get the lo residual pass (rel err
KO_LO = KO - KO_LO_START  # ~1.76e-2 vs 1.66e-2 at 16, saves 1 DR MM/group)
N_TILE = 512

F32 = mybir.dt.float32
BF16 = mybir.dt.bfloat16
FP8 = mybir.dt.float8e4
DR = mybir.MatmulPerfMode.DoubleRow


def build(m_shard=M_SHARD, d_in=D_IN, d_out=N_SHARD, reps=1, mode="full",
          wstage_bufs=3, wchunk=4, outs_bufs=4):
    """mode: 'full' (real kernel), 'pe' (timing probe: no input DMA/quant),
    'dma' (W DMA + quant, no matmuls), 'dmax' (x DMA only)."""
    ko = d_in // P
    ko_lo_start = (ko * KO_LO_START) // KO
    ko_lo = ko - ko_lo_start
    n_tiles = d_out // N_TILE
    m_sub = m_shard // P
    xg = max(1, m_shard // 256)
    xw = m_shard // xg

    nc = bacc.Bacc("TRN2", target_bir_lowering=False, debug=False,
                   num_devices=N_CORES)
    xt_d = nc.declare_dram_parameter("xt", [P, ko, m_shard], F32, isOutput=False)
    wt_d = nc.declare_dram_parameter("wt", [P, ko, d_out], F32, isOutput=False)
    al_d = nc.declare_dram_parameter("alpha", [1, 1], F32, isOutput=False)
    out_d = nc.declare_dram_parameter("out", [P, m_sub, d_out], F32, isOutput=True)

    with tile.TileContext(nc) as tc:
        with (
            tc.tile_pool(name="const", bufs=1) as const,
            tc.tile_pool(name="xres", bufs=1) as xres_pool,
            tc.tile_pool(name="stage", bufs=2) as stage,
            tc.tile_pool(name="wstage", bufs=wstage_bufs) as wstage,
            tc.tile_pool(name="wq", bufs=n_tiles) as wqp,
            tc.tile_pool(name="s2", bufs=2) as s2p,
            tc.tile_pool(name="outs", bufs=outs_bufs) as outs,
            tc.tile_pool(name="psum", bufs=8, space="PSUM") as psum,
        ):
            pe_init = {}
            if mode == "pe":
                pe_init["xres"] = [
                    xres_pool.tile([P, ko, xw], FP8, tag=f"xres{g}",
                                   name=f"xres{g}")
                    for g in range(xg)
                ]
                pe_init["xlo"] = [
                    xres_pool.tile([P, ko_lo, xw], FP8, tag=f"xlo{g}",
                                   name=f"xlo{g}")
                    for g in range(xg)
                ]
                for g in range(xg):
                    nc.vector.memset(pe_init["xres"][g][:], 0.0)
                    nc.vector.memset(pe_init["xlo"][g][:], 0.0)
                pe_init["wq"] = wqp.tile([P, ko, N_TILE], FP8, tag="wq",
                                         name="wq_static")
                nc.vector.memset(pe_init["wq"][:], 0.0)

            rep_ctx = (
                tc.For_i(0, reps, 1) if reps > 1 else contextlib.nullcontext()
            )
            with rep_ctx:
                # alpha -> [1,1] -> broadcast to [128,1]; +a/2 and -a/2.
                a1 = const.tile([1, 1], F32)
                nc.sync.dma_start(out=a1[:, :], in_=al_d.ap()[:, :])
                ab = const.tile([P, 1], F32)
                nc.gpsimd.partition_broadcast(ab[:, :], a1[:, :])
                half = const.tile([P, 1], F32)
                nc.vector.tensor_scalar_mul(half[:, :], ab[:, :], 0.5)
                neghalf = const.tile([P, 1], F32)
                nc.vector.tensor_scalar_mul(neghalf[:, :], ab[:, :], -0.5)

                # x^T shard resident in SBUF as fp8 hi (+ lo residual for
                # the upper half of K), xg column groups.
                if mode == "pe":
                    xres = pe_init["xres"]
                    xlo = pe_init["xlo"]
                else:
                    xres = [
                        xres_pool.tile([P, ko, xw], FP8, tag=f"xres{g}",
                                       name=f"xres{g}")
                        for g in range(xg)
                    ]
                    xlo = [
                        xres_pool.tile([P, ko_lo, xw], FP8, tag=f"xlo{g}",
                                       name=f"xlo{g}")
                        for g in range(xg)
                    ]

                def load_x_group(g, casts=True):
                    for k4 in range(ko // 4):
                        st = stage.tile([P, 4, xw], F32, tag="xstage")
                        nc.sync.dma_start(
                            out=st[:, :, :],
                            in_=xt_d.ap()[:, k4 * 4:(k4 + 1) * 4,
                                          g * xw:(g + 1) * xw],
                        )
                        if not casts:
                            continue
                        hi = xres[g][:, k4 * 4:(k4 + 1) * 4, :]
                        nc.vector.tensor_copy(hi, st[:, :, :])
                        s0 = max(k4 * 4, ko_lo_start)
                        if s0 < (k4 + 1) * 4:
                            # lo = x - hi for k-tiles >= ko_lo_start
                            nc.vector.tensor_tensor(
                                xlo[g][:, s0 - ko_lo_start:
                                       (k4 + 1) * 4 - ko_lo_start, :],
                                st[:, s0 - k4 * 4:, :],
                                xres[g][:, s0:(k4 + 1) * 4, :],
                                mybir.AluOpType.subtract,
                            )

                if mode in ("xonly", "dmax"):
                    for g in range(xg):
                        load_x_group(g, casts=(mode == "xonly"))
                    wq0 = None

                def make_wq(n):
                    # Stream + ternarize one n-tile's W^T panel into ONE
                    # resident fp8 tile. Per-chunk sign/sign/add so the
                    # panel becomes usable k-chunk by k-chunk (short head,
                    # fine-grained PE gating via Tile semaphores).
                    if mode == "pe":
                        return pe_init["wq"]
                    wq = wqp.tile([P, ko, N_TILE], FP8, tag="wq", name="wq")
                    for c in range(ko // wchunk):
                        st = wstage.tile([P, wchunk, N_TILE], F32,
                                         tag="wst")
                        nc.sync.dma_start(
                            out=st[:, :, :],
                            in_=wt_d.ap()[:, c * wchunk:(c + 1) * wchunk,
                                          n * N_TILE:(n + 1) * N_TILE],
                        )
                        qs = wq[:, c * wchunk:(c + 1) * wchunk, :]
                        s2 = s2p.tile([P, wchunk, N_TILE], FP8, tag="s2",
                                      name="s2")
                        nc.scalar.sign(qs, st[:, :, :], bias=half[:, :])
                        nc.scalar.sign(s2[:, :, :], st[:, :, :],
                                       bias=neghalf[:, :])
                        nc.vector.tensor_tensor(
                            qs, qs, s2[:, :, :], mybir.AluOpType.add
                        )
                    return wq

                mg = xw // P  # m-groups per xres tile
                wq_tiles = {}

                def emit_cells(cells):
                    # cells: list of (panel n, x-group g). Emission is
                    # k-MAJOR across all psum groups of the batch: each
                    # arriving W/x chunk (4 k-tiles) enables 2 DR MMs on
                    # EVERY open group, so during the DMA-bound front the
                    # PE advances at (2 x n_groups) MMs per chunk instead
                    # of head-of-line-blocking on one crawling cell.
                    # Max batch = 4 cells x mg=2 -> 8 psum banks exactly.
                    groups = []
                    for n, g in cells:
                        for col in range(mg):
                            ps = psum.tile([P, N_TILE], F32, tag="ps",
                                           name="ps")
                            groups.append((ps, n, g, col))
                    for c in range(ko // 4):
                        for ps, n, g, col in groups:
                            cs = slice(col * P, (col + 1) * P)
                            for kg in (2 * c, 2 * c + 1):
                                # hi pass: DoubleRow over k-tile pair
                                nc.tensor.matmul(
                                    ps[:, :],
                                    lhsT=xres[g][:, 2 * kg:2 * kg + 2, cs],
                                    rhs=wq_tiles[n][:, 2 * kg:2 * kg + 2, :],
                                    start=(kg == 0),
                                    stop=False,
                                    perf_mode=DR,
                                )
                    for ps, n, g, col in groups:
                        cs = slice(col * P, (col + 1) * P)
                        for kg in range(ko_lo // 2):
                            # lo pass: residual for k-tiles >= ko_lo_start
                            nc.tensor.matmul(
                                ps[:, :],
                                lhsT=xlo[g][:, 2 * kg:2 * kg + 2, cs],
                                rhs=wq_tiles[n][:, ko_lo_start + 2 * kg:
                                               ko_lo_start + 2 * kg + 2, :],
                                start=False,
                                stop=(kg == ko_lo // 2 - 1),
                                perf_mode=DR,
                            )
                        ot = outs.tile([P, N_TILE], F32, tag="ot",
                                       name="ot")
                        # out = psum * (alpha/2): undoes the {-2,0,2}
                        # doubling and applies the alpha weight scale.
                        nc.scalar.mul(ot[:, :], ps[:, :], half[:, :])
                        # ACT's HWDGE ring, so output stores don't
                        # queue behind the input stream on SP's ring.
                        nc.scalar.dma_start(
                            out=out_d.ap()[:, g * mg + col,
                                           n * N_TILE:(n + 1) * N_TILE],
                            in_=ot[:, :],
                        )

                # Wavefront schedule: all W panels stay resident as fp8
                # (8.4 MB total), x groups stream in; after every arrival
                # (x group or W panel) emit all newly-enabled cells. The
                # enabled PE work grows with loaded_x * loaded_W, so the
                # oversubscribed DMA front never starves the PE for long.
                if mode == "pe":
                    for n in range(n_tiles):
                        wq_tiles[n] = pe_init["wq"]
                    for n in range(n_tiles):
                        for g in range(xg):
                            emit_cells([(n, g)])
                elif mode == "dma":
                    for n in range(n_tiles):
                        wq_tiles[n] = make_wq(n)
                elif mode in ("xonly", "dmax"):
                    for g in range(xg):
                        load_x_group(g, casts=(mode == "xonly"))
                else:
                    if n_tiles == 4 and xg == 8:
                        # Hand-tuned arrival order (x group = 4.2 MB,
                        # W panel = 8.4 MB on the same DMA ring).
                        events = [("x", 0), ("w", 0), ("w", 1), ("x", 1),
                                  ("x", 2), ("w", 2), ("x", 3), ("w", 3)]
                        events += [("x", i) for i in range(4, xg)]
                    else:
                        events = [("x", 0), ("w", 0)]
                        k = 1
                        while k < max(n_tiles, xg):
                            if k < n_tiles:
                                events.append(("w", k))
                            if k < xg:
                                events.append(("x", k))
                            k += 1
                    loaded_g, loaded_w = [], []
                    for kind, idx in events:
                        if kind == "x":
                            load_x_group(idx)
                            loaded_g.append(idx)
                            batch = [(n, idx) for n in loaded_w]
                        else:
                            wq_tiles[idx] = make_wq(idx)
                            loaded_w.append(idx)
                            batch = [(idx, g) for g in loaded_g]
                        # One k-major batch per arrival; batches with >4
                        # cells would exceed the 8 PSUM banks, so split.
                        for i in range(0, len(batch), 4):
                            emit_cells(batch[i:i + 4])

    nc.compile()
    return nc


_NC_CACHE = {}


def _get_nc():
    if "nc" not in _NC_CACHE:
        _NC_CACHE["nc"] = build()
    return _NC_CACHE["nc"]


def make_in_maps(x, W, alpha):
    x = np.ascontiguousarray(np.asarray(x, np.float32)).reshape(M_TOT, D_IN)
    W = np.ascontiguousarray(np.asarray(W, np.float32))
    a = np.full((1, 1), np.float32(np.asarray(alpha)), np.float32)
    # Per col-group: wt[p, k, n] = W[cg*N_SHARD + n, k*128 + p]
    wts = []
    for cg in range(CG):
        ws = W[cg * N_SHARD:(cg + 1) * N_SHARD]
        wts.append(np.ascontiguousarray(
            ws.reshape(N_SHARD, KO, P).transpose(2, 1, 0)))
    # Per row-group: xt[p, k, m] = xs[m, k*128 + p]
    xts = []
    for rg in range(RG):
        xs = x[rg * M_SHARD:(rg + 1) * M_SHARD]
        xts.append(np.ascontiguousarray(
            xs.reshape(M_SHARD, KO, P).transpose(2, 1, 0)))
    in_maps = []
    for c in range(N_CORES):
        rg, cg = divmod(c, CG)
        in_maps.append({"xt": xts[rg], "wt": wts[cg], "alpha": a})
    return in_maps


def gather_out(results):
    m_sub = M_SHARD // P
    full = np.empty((M_TOT, D_OUT), np.float32)
    for c in range(N_CORES):
        rg, cg = divmod(c, CG)
        o = results[c]["out"]  # [P, m_sub, N_SHARD]; row = mo*128 + p
        full[rg * M_SHARD:(rg + 1) * M_SHARD,
             cg * N_SHARD:(cg + 1) * N_SHARD] = (
            o.transpose(1, 0, 2).reshape(M_SHARD, N_SHARD))
    return full.reshape(4, 2048, D_OUT)


def kernel(x, W, alpha):
    nc = _get_nc()
    in_maps = make_in_maps(x, W, alpha)
    res = run_bass_kernel_spmd(nc, in_maps, core_ids=list(range(N_CORES)))
    return gather_out(res.results)
